# revision 33
# baseline (speedup 1.0000x reference)
"""CLoRALinear Trainium2 kernel (bf16 + fp8-DoubleRow hybrid).

Computes y = x @ (W + (alpha/r) * A @ B.T).T + bias for
x:[4,2048,4096] f32, W:[4096,4096], bias:[4096], A:[4096,32], B:[4096,32].

Strategy: data-parallel over tokens across 8 NeuronCores (1024 tokens each).
Per core the contraction dim (4096 = 32 k-tiles of 128) is split:
  k-tiles  0..KTB-1  : bf16 matmuls (fp32 PSUM accum)
  k-tiles KTB..31    : fp8e4 DoubleRow matmuls (2 k-tiles per instruction,
                       2x PE throughput; measured 216ns per DR instr = same
                       as one bf16 instr for twice the K)
The fp8 fraction (10/32) puts the end-to-end rel err at ~1.78e-2, under the
2e-2 gate.  W (std 0.02) would be subnormal in e4m3, so the fp8 path carries
a x256 scale applied by the W.T PSUM->SBUF copies (PE transposes are pure
permutations and cannot scale); PSUM = 256*y and the y copy-out is an ACT
copy with scale 1/256.  B is likewise scaled x256 (u = 256*x@B), and A/bias
enter via the augmented LoRA matmul [u ; 1 ; 0pad] @ [A.T ; 256*bias ; 0],
zero-padded to K=128 -- a K=33 matmul forces a PE geometry switch costing
~320ns per output tile.

x.T and W.T tiles are produced on-chip by PE transposes (fp32 inputs have no
DMA-transpose path; fp32->bf16 casts ride the SWDGE loads).  The startup is
DMA-bandwidth-bound (x + W slices 0/1 + B/A in the first ~70us), so: B/A
gathers are quartered across SWDGE rings, slices 0 and 1 are processed in
interleaved half-slices (m 0-3 of each, then m 4-7) to give the tail x
chunks ~100us of arrival slack, and x transposes/u batches sit as late as
dependencies allow so a lagging DMA never blocks a ready main tile.  W.T
transposes for the next slice are interleaved after each m-tile's matmul
group, batched 4-8-per-PSUM-bank with a single copy out (alternating
DVE/ACT) so copies never gate the PE.
"""

import sys

sys.path.insert(0, "/opt/trn_rl_repo")

import numpy as np

import concourse.bass as bass
import concourse.tile as tile
from concourse import bacc, mybir
from concourse.bass_utils import run_bass_kernel_spmd
from concourse.masks import make_identity

F32 = mybir.dt.float32
BF16 = mybir.dt.bfloat16
FP8 = mybir.dt.float8e4
DR = mybir.MatmulPerfMode.DoubleRow

N_CORES = 8
TOK = 1024          # tokens per core
DIN = 4096
DOUT = 4096
R = 32
KT = DIN // 128     # 32 k-tiles
KT8 = 10            # fp8 k-tiles (last KT8 of KT; must be even)
KTB = KT - KT8      # bf16 k-tiles
MT = TOK // 128     # 8 m-tiles
NSL = 512           # out-features per n-slice
NT = DOUT // NSL    # 8 n-slices
CPS = NSL // 128    # 4 weight chunks per n-slice
SW = 256.0          # fp8/W scale (power of two; PSUM holds 256*y)

_cached = None


def _build():
    nc = bacc.Bacc("TRN2", target_bir_lowering=False, debug=False)

    x_d = nc.dram_tensor("x", [TOK, DIN], F32, kind="ExternalInput").ap()
    w_d = nc.dram_tensor("weight", [DOUT, DIN], F32, kind="ExternalInput").ap()
    bias_d = nc.dram_tensor("bias", [DOUT], F32, kind="ExternalInput").ap()
    a_d = nc.dram_tensor("A", [DOUT, R], F32, kind="ExternalInput").ap()
    b_d = nc.dram_tensor("B", [DIN, R], F32, kind="ExternalInput").ap()
    y_d = nc.dram_tensor("out", [TOK, DOUT], F32, kind="ExternalOutput").ap()

    with tile.TileContext(nc) as tc:
        with (
            tc.tile_pool(name="const", bufs=1) as const_pool,
            tc.tile_pool(name="xchunk", bufs=2) as xchunk_pool,
            tc.tile_pool(name="wchunk", bufs=5) as wchunk_pool,
            tc.tile_pool(name="wT", bufs=2) as wT_pool,
            tc.tile_pool(name="yout", bufs=3) as y_pool,
            tc.tile_pool(name="tpsum", bufs=6, space="PSUM") as tpsum_pool,
            tc.tile_pool(name="ypsum", bufs=2, space="PSUM") as ypsum_pool,
        ):
            ident = const_pool.tile([128, 128], BF16)
            make_identity(nc, ident[:])
            ident_f32 = const_pool.tile([128, 128], F32)

            copy_idx = [0]

            def tcopy(dst, src):
                if copy_idx[0] % 2 == 0:
                    nc.vector.tensor_copy(dst, src)
                else:
                    nc.scalar.copy(dst, src)
                copy_idx[0] += 1

            def tcopy_scaled(dst, src, scale):
                # W.T copy-outs carry the fp8 x256 scale (PE transposes are
                # pure permutations, so the scale must ride the copy)
                if copy_idx[0] % 2 == 0:
                    nc.vector.tensor_scalar_mul(dst, src, scale)
                else:
                    nc.scalar.mul(dst, src, scale)
                copy_idx[0] += 1

            # u_aug/a_aug are zero-padded to 128 partitions (logical rows:
            # 0:R = LoRA rank, R = ones/bias, rest zero) so the LoRA matmul
            # keeps the PE at K=128 geometry -- a K=33 matmul costs ~+320ns
            # per tile in PE reconfig.
            x_t = const_pool.tile([128, KTB, TOK], BF16)
            x8_t = const_pool.tile([128, KT8, TOK], FP8)
            u_aug = const_pool.tile([128, TOK], BF16)
            b_all = const_pool.tile([128, KT, R], BF16)
            b8_all = const_pool.tile([128, KT8, R], FP8)

            # ---- kick off startup DMAs ----
            x_chunks = {}

            def load_x_chunk(m):
                # quartered across DMA rings: x chunks are the latency-
                # critical stream during the n=0 window
                if m % 2 == 0:
                    ch = xchunk_pool.tile(
                        [128, DIN], BF16, tag="xchunk", name="x_chunk"
                    )
                    eng = nc.gpsimd
                else:
                    ch = xchunk_pool.tile(
                        [128, DIN], F32, tag="xf32", name="x_chunk", bufs=1
                    )
                    eng = nc.sync
                for q in range(2):
                    cs = slice(q * DIN // 2, (q + 1) * DIN // 2)
                    eng.dma_start(ch[:, cs], x_d[m * 128:(m + 1) * 128, cs])
                x_chunks[m] = ch

            w_chunks = {}

            def load_w_chunk(n, c):
                ch = wchunk_pool.tile([128, DIN], BF16, tag="wchunk")
                nc.gpsimd.dma_start(
                    ch[:], w_d[n * NSL + c * 128:n * NSL + (c + 1) * 128, :]
                )
                w_chunks[(n, c)] = ch

            # bias first on the SWDGE queue (tiny cast-DMA), then the bulk
            # chunk loads; B/A ride the sync HW queue as f32 (the SWDGE
            # software gather takes ~45us and stalled the PE for 25us)
            a_aug = const_pool.tile([128, DOUT], BF16)
            for p0 in range(R, 128, 32):
                nc.gpsimd.memset(a_aug[p0:p0 + 32, :], 0.0)
            nc.gpsimd.dma_start(a_aug[R:R + 1, :], bias_d[None, :])
            load_x_chunk(0)
            load_x_chunk(1)
            # B/A gathers: quartered across SWDGE rings so they finish in
            # ~1/4 the time (a single gather ran ~25-45us and stalled the
            # PE before the first u batch); issued before the W chunks.
            b_all_nat = b_d.rearrange("(k p) r -> p k r", p=128)
            a_all_nat = a_d.rearrange("(o p) r -> p o r", p=128)
            b_nat32 = const_pool.tile([128, KT, R], F32)
            a_nat32 = const_pool.tile([128, DOUT // 128, R], F32)
            for q in range(4):
                ks = slice(q * KT // 4, (q + 1) * KT // 4)
                nc.gpsimd.dma_start(b_nat32[:, ks, :], b_all_nat[:, ks, :])
            for q in range(4):
                ks = slice(q * 8, (q + 1) * 8)
                nc.gpsimd.dma_start(a_nat32[:, ks, :], a_all_nat[:, ks, :])
            for c in range(CPS):
                load_w_chunk(0, c)

            make_identity(nc, ident_f32[:])
            nc.gpsimd.memset(u_aug[:], 0.0)
            nc.gpsimd.memset(u_aug[R:R + 1, :], 1.0)

            w_t = [
                wT_pool.tile([128, KTB, NSL], BF16, tag="wt", name="wt0"),
                wT_pool.tile([128, KTB, NSL], BF16, tag="wt", name="wt1"),
            ]
            w8_t = [
                wT_pool.tile([128, KT8, NSL], FP8, tag="wt8", name="w8t0"),
                wT_pool.tile([128, KT8, NSL], FP8, tag="wt8", name="w8t1"),
            ]

            def split_ranges(b0, bs):
                """Split block range [b0, b0+bs) at the KTB dtype boundary."""
                if b0 >= KTB or b0 + bs <= KTB:
                    return [(b0, b0 + bs)]
                return [(b0, KTB), (KTB, b0 + bs)]

            def w_transpose_run(n, c, k0, nk):
                """Transpose nk k-blocks (k indices k0..) of chunk c of slice
                n into w_t (bf16, k<KTB) / w8_t (fp8, k>=KTB), batching one
                full PSUM bank per copy; copies apply the x256 fp8 scale."""
                ch = w_chunks[(n, c)]
                f32 = ch.dtype == F32
                bs = 4 if f32 else 8
                idn = ident_f32 if f32 else ident
                col0 = c * 128
                for b0 in range(k0, k0 + nk, bs):
                    pt = tpsum_pool.tile(
                        [128, bs, 128], F32 if f32 else BF16, tag="t"
                    )
                    for j in range(bs):
                        nc.tensor.transpose(
                            pt[:, j, :],
                            ch[:, (b0 + j) * 128:(b0 + j + 1) * 128],
                            idn[:],
                        )
                    for r0, r1 in split_ranges(b0, bs):
                        if r0 >= KTB:
                            dst = w8_t[n % 2][:, r0 - KTB:r1 - KTB,
                                              col0:col0 + 128]
                        else:
                            dst = w_t[n % 2][:, r0:r1, col0:col0 + 128]
                        tcopy_scaled(dst, pt[:, r0 - b0:r1 - b0, :], SW)

            def x_transpose_run(m):
                """Transpose all KT k-blocks of x m-chunk: k<KTB into x_t
                (bf16), k>=KTB into x8_t (fp8, unscaled cast)."""
                chunk = x_chunks.pop(m)
                f32 = chunk.dtype == F32
                bs = 4 if f32 else 8
                idn = ident_f32 if f32 else ident
                col0 = m * 128
                for b0 in range(0, KT, bs):
                    pt = tpsum_pool.tile(
                        [128, bs, 128], F32 if f32 else BF16, tag="t"
                    )
                    for j in range(bs):
                        nc.tensor.transpose(
                            pt[:, j, :],
                            chunk[:, (b0 + j) * 128:(b0 + j + 1) * 128],
                            idn[:],
                        )
                    for r0, r1 in split_ranges(b0, bs):
                        if r0 >= KTB:
                            dst = x8_t[:, r0 - KTB:r1 - KTB, col0:col0 + 128]
                        else:
                            dst = x_t[:, r0:r1, col0:col0 + 128]
                        tcopy(dst, pt[:, r0 - b0:r1 - b0, :])

            def u_batch(mc):
                """u[:, mc*256:(mc+1)*256] = 256*(x @ B).T for 2 m-tiles."""
                cols = slice(mc * 256, (mc + 1) * 256)
                up = ypsum_pool.tile([R, 256], F32, tag="y", name="up")
                for k in range(KTB):
                    nc.tensor.matmul(
                        up[:], b_all[:, k, :], x_t[:, k, cols],
                        start=(k == 0), stop=False,
                    )
                for kk in range(KT8 // 2):
                    nc.tensor.matmul(
                        up[:],
                        b8_all[:, 2 * kk:2 * kk + 2, :],
                        x8_t[:, 2 * kk:2 * kk + 2, cols],
                        start=False, stop=(kk == KT8 // 2 - 1),
                        perf_mode=DR, skip_group_check=True,
                    )
                tcopy(u_aug[0:R, cols], up[:])

            def main_tile(n, m, before_lora=None):
                """One [128 tok, 512 out] output tile: bf16 k-loop + fp8-DR
                k-loop + augmented LoRA matmul, then scaled copy-out + DMA.
                before_lora emits extra PE work between the k-loop and the
                LoRA matmul (startup uses this to cover the A gather)."""
                cur = w_t[n % 2]
                cur8 = w8_t[n % 2]
                yp = ypsum_pool.tile([128, NSL], F32, tag="y")
                for k in range(KTB):
                    nc.tensor.matmul(
                        yp[:], x_t[:, k, m * 128:(m + 1) * 128], cur[:, k, :],
                        start=(k == 0), stop=False,
                    )
                for kk in range(KT8 // 2):
                    nc.tensor.matmul(
                        yp[:],
                        x8_t[:, 2 * kk:2 * kk + 2, m * 128:(m + 1) * 128],
                        cur8[:, 2 * kk:2 * kk + 2, :],
                        start=False, stop=False,
                        perf_mode=DR, skip_group_check=True,
                    )
                if before_lora is not None:
                    before_lora()
                nc.tensor.matmul(
                    yp[:],
                    u_aug[:, m * 128:(m + 1) * 128],
                    a_aug[:, n * NSL:(n + 1) * NSL],
                    start=False, stop=True, skip_group_check=True,
                )
                y_sb = y_pool.tile([128, NSL], F32, tag="ysb")
                nc.scalar.mul(y_sb[:], yp[:], 1.0 / SW)
                nc.sync.dma_start(
                    y_d[m * 128:(m + 1) * 128, n * NSL:(n + 1) * NSL],
                    y_sb[:],
                )

            # ---- startup: x m=0,1 + W slice 0 + B/A constants ----
            nc.vector.tensor_scalar_mul(
                a_aug[R:R + 1, :], a_aug[R:R + 1, :], SW
            )
            x_transpose_run(0)
            load_x_chunk(2)
            x_transpose_run(1)
            load_x_chunk(3)
            load_w_chunk(1, 0)
            w_transpose_run(0, 0, 0, 16)
            w_transpose_run(0, 0, 16, 16)
            w_transpose_run(0, 1, 0, 16)
            # B: x256 scale + bf16 cast, + fp8 cast for the fp8 k-range
            # (emitted mid-wT0 so a late B gather can't block the wT0
            # PSUM-copy chain on DVE/ACT)
            nc.vector.tensor_scalar_mul(b_all[:], b_nat32[:], SW)
            nc.scalar.copy(b8_all[:], b_all[:, KTB:, :])
            w_transpose_run(0, 1, 16, 16)
            w_transpose_run(0, 2, 0, 16)
            w_transpose_run(0, 2, 16, 16)
            w_transpose_run(0, 3, 0, 16)
            w_transpose_run(0, 3, 16, 16)
            load_w_chunk(1, 1)
            # W slice-1 chunk-0 transposes: PE cover for the B gather+casts
            w_transpose_run(1, 0, 0, 16)
            w_transpose_run(1, 0, 16, 16)
            u_batch(0)

            def build_a_aug():
                # A_aug rows 0:R = A.T (f32 transposes, cast on copy-out);
                # runs inside tile (0,0) so the k-loop covers the A gather
                for o in range(DOUT // 128):
                    pt = tpsum_pool.tile([R, 128], F32, tag="t")
                    nc.tensor.transpose(pt[:], a_nat32[:, o, :], ident_f32[:])
                    nc.vector.tensor_copy(
                        a_aug[0:R, o * 128:(o + 1) * 128], pt[:]
                    )

            main_tile(0, 0, before_lora=build_a_aug)

            # ---- slices 0 and 1 interleaved in half-slices so the tail x
            # chunks (m=4..7) get ~100us of DMA slack instead of ~10us ----
            # phase A: tiles (0, 1..3) + slice-1 transposes
            for m in range(1, 4):
                if m == 1:
                    load_w_chunk(1, 2)
                    load_w_chunk(1, 3)
                main_tile(0, m)
                w_transpose_run(1, m, 0, 16)
                w_transpose_run(1, m, 16, 16)
                if m == 1:
                    x_transpose_run(2)
                    load_x_chunk(4)
                    x_transpose_run(3)
                    load_x_chunk(5)
                    u_batch(1)
            # phase B: tiles (1, 0..3) + slice-2 chunk loads
            for m in range(4):
                load_w_chunk(2, m)
                main_tile(1, m)
                if m == 0:
                    x_transpose_run(4)
                    load_x_chunk(6)
                if m == 1:
                    x_transpose_run(5)
                    load_x_chunk(7)
                    u_batch(2)
            # phase C: tiles (0, 4..7)
            for m in range(4, MT):
                main_tile(0, m)
                if m == 4:
                    x_transpose_run(6)
                if m == 5:
                    x_transpose_run(7)
                    u_batch(3)
            # phase D: tiles (1, 4..7) + slice-2 transposes
            for m in range(4, MT):
                main_tile(1, m)
                w_transpose_run(2, m - 4, 0, 16)
                w_transpose_run(2, m - 4, 16, 16)

            # ---- main loop over remaining output-feature slices ----
            for n in range(2, NT):
                for m in range(MT):
                    if n + 1 < NT:
                        # chunk c of slice n+1 is consumed at m=2c and 2c+1;
                        # load it one m-iteration ahead (c=0 at m=0).
                        if m == 0:
                            load_w_chunk(n + 1, 0)
                        if m % 2 == 1 and (m + 1) // 2 < CPS:
                            load_w_chunk(n + 1, (m + 1) // 2)
                    main_tile(n, m)
                    if n + 1 < NT:
                        # 16 transposes of slice n+1 after each m's matmuls
                        w_transpose_run(n + 1, m // 2, (m % 2) * 16, 16)

    nc.compile()
    return nc


def _get_nc():
    global _cached
    if _cached is None:
        _cached = _build()
    return _cached


def kernel(x, weight, bias, A, B, _trace=False):
    x = np.ascontiguousarray(np.asarray(x, dtype=np.float32)).reshape(-1, DIN)
    weight = np.ascontiguousarray(np.asarray(weight, dtype=np.float32))
    bias = np.ascontiguousarray(np.asarray(bias, dtype=np.float32))
    A = np.ascontiguousarray(np.asarray(A, dtype=np.float32))
    B = np.ascontiguousarray(np.asarray(B, dtype=np.float32))

    nc = _get_nc()
    in_maps = [
        {
            "x": np.ascontiguousarray(x[c * TOK:(c + 1) * TOK]),
            "weight": weight,
            "bias": bias,
            "A": A,
            "B": B,
        }
        for c in range(N_CORES)
    ]
    # the axon trn2 runtime very occasionally hard-faults a core
    # (NRT_EXEC_UNIT_UNRECOVERABLE) on a fresh load; retry once
    last_exc = None
    for attempt in range(3):
        try:
            res = run_bass_kernel_spmd(
                nc, in_maps, core_ids=list(range(N_CORES)), trace=_trace
            )
            kernel.last_result = res
            y = np.concatenate(
                [np.asarray(res.results[c]["out"]) for c in range(N_CORES)],
                axis=0,
            )
            return y.reshape(4, 2048, DOUT)
        except Exception as exc:  # noqa: BLE001
            last_exc = exc
            import time as _time

            _time.sleep(10 * (attempt + 1))
    raise last_exc


kernel.last_result = None


# revision 34
# speedup vs baseline: 1.0175x; 1.0175x over previous
"""CLoRALinear Trainium2 kernel (bf16 + fp8-DoubleRow hybrid).

Computes y = x @ (W + (alpha/r) * A @ B.T).T + bias for
x:[4,2048,4096] f32, W:[4096,4096], bias:[4096], A:[4096,32], B:[4096,32].

Strategy: data-parallel over tokens across 8 NeuronCores (1024 tokens each).
Per core the contraction dim (4096 = 32 k-tiles of 128) is split:
  k-tiles  0..KTB-1  : bf16 matmuls (fp32 PSUM accum)
  k-tiles KTB..31    : fp8e4 DoubleRow matmuls (2 k-tiles per instruction,
                       2x PE throughput; measured 216ns per DR instr = same
                       as one bf16 instr for twice the K)
The fp8 fraction (10/32) puts the end-to-end rel err at ~1.78e-2, under the
2e-2 gate.  W (std 0.02) would be subnormal in e4m3, so the fp8 path carries
a x256 scale applied by the W.T PSUM->SBUF copies (PE transposes are pure
permutations and cannot scale); PSUM = 256*y and the y copy-out is an ACT
copy with scale 1/256.  B is likewise scaled x256 (u = 256*x@B), and A/bias
enter via the augmented LoRA matmul [u ; 1 ; 0pad] @ [A.T ; 256*bias ; 0],
zero-padded to K=128 -- a K=33 matmul forces a PE geometry switch costing
~320ns per output tile.

x.T and W.T tiles are produced on-chip by PE transposes (fp32 inputs have no
DMA-transpose path; fp32->bf16 casts ride the SWDGE loads).  The startup is
DMA-bandwidth-bound (x + W slices 0/1 + B/A in the first ~70us), so: B/A
gathers are quartered across SWDGE rings, slices 0 and 1 are processed in
interleaved half-slices (m 0-3 of each, then m 4-7) to give the tail x
chunks ~100us of arrival slack, and x transposes/u batches sit as late as
dependencies allow so a lagging DMA never blocks a ready main tile.  W.T
transposes for the next slice are interleaved after each m-tile's matmul
group, batched 4-8-per-PSUM-bank with a single copy out (alternating
DVE/ACT) so copies never gate the PE.
"""

import sys

sys.path.insert(0, "/opt/trn_rl_repo")

import numpy as np

import concourse.bass as bass
import concourse.tile as tile
from concourse import bacc, mybir
from concourse.bass_utils import run_bass_kernel_spmd
from concourse.masks import make_identity

F32 = mybir.dt.float32
BF16 = mybir.dt.bfloat16
FP8 = mybir.dt.float8e4
DR = mybir.MatmulPerfMode.DoubleRow

N_CORES = 8
TOK = 1024          # tokens per core
DIN = 4096
DOUT = 4096
R = 32
KT = DIN // 128     # 32 k-tiles
KT8 = 10            # fp8 k-tiles (last KT8 of KT; must be even)
KTB = KT - KT8      # bf16 k-tiles
MT = TOK // 128     # 8 m-tiles
NSL = 512           # out-features per n-slice
NT = DOUT // NSL    # 8 n-slices
CPS = NSL // 128    # 4 weight chunks per n-slice
SW = 256.0          # fp8/W scale (power of two; PSUM holds 256*y)

_cached = None


def _build():
    nc = bacc.Bacc("TRN2", target_bir_lowering=False, debug=False)

    x_d = nc.dram_tensor("x", [TOK, DIN], F32, kind="ExternalInput").ap()
    w_d = nc.dram_tensor("weight", [DOUT, DIN], F32, kind="ExternalInput").ap()
    bias_d = nc.dram_tensor("bias", [DOUT], F32, kind="ExternalInput").ap()
    a_d = nc.dram_tensor("A", [DOUT, R], F32, kind="ExternalInput").ap()
    b_d = nc.dram_tensor("B", [DIN, R], F32, kind="ExternalInput").ap()
    y_d = nc.dram_tensor("out", [TOK, DOUT], F32, kind="ExternalOutput").ap()

    with tile.TileContext(nc) as tc:
        with (
            tc.tile_pool(name="const", bufs=1) as const_pool,
            tc.tile_pool(name="xchunk", bufs=2) as xchunk_pool,
            tc.tile_pool(name="wchunk", bufs=5) as wchunk_pool,
            tc.tile_pool(name="wT", bufs=2) as wT_pool,
            tc.tile_pool(name="yout", bufs=3) as y_pool,
            tc.tile_pool(name="tpsum", bufs=6, space="PSUM") as tpsum_pool,
            tc.tile_pool(name="ypsum", bufs=2, space="PSUM") as ypsum_pool,
        ):
            ident = const_pool.tile([128, 128], BF16)
            make_identity(nc, ident[:])
            ident_f32 = const_pool.tile([128, 128], F32)

            copy_idx = [0]

            def tcopy(dst, src):
                if copy_idx[0] % 2 == 0:
                    nc.vector.tensor_copy(dst, src)
                else:
                    nc.scalar.copy(dst, src)
                copy_idx[0] += 1

            def tcopy_scaled(dst, src, scale):
                # W.T copy-outs carry the fp8 x256 scale (PE transposes are
                # pure permutations, so the scale must ride the copy)
                if copy_idx[0] % 2 == 0:
                    nc.vector.tensor_scalar_mul(dst, src, scale)
                else:
                    nc.scalar.mul(dst, src, scale)
                copy_idx[0] += 1

            # u_aug/a_aug are zero-padded to 128 partitions (logical rows:
            # 0:R = LoRA rank, R = ones/bias, rest zero) so the LoRA matmul
            # keeps the PE at K=128 geometry -- a K=33 matmul costs ~+320ns
            # per tile in PE reconfig.
            x_t = const_pool.tile([128, KTB, TOK], BF16)
            x8_t = const_pool.tile([128, KT8, TOK], FP8)
            u_aug = const_pool.tile([128, TOK], BF16)
            b_all = const_pool.tile([128, KT, R], BF16)
            b8_all = const_pool.tile([128, KT8, R], FP8)

            # ---- kick off startup DMAs ----
            x_chunks = {}

            def load_x_chunk(m):
                # quartered across DMA rings: x chunks are the latency-
                # critical stream during the n=0 window
                if m % 2 == 0:
                    ch = xchunk_pool.tile(
                        [128, DIN], BF16, tag="xchunk", name="x_chunk"
                    )
                    eng = nc.gpsimd
                else:
                    ch = xchunk_pool.tile(
                        [128, DIN], F32, tag="xf32", name="x_chunk", bufs=1
                    )
                    eng = nc.sync
                for q in range(2):
                    cs = slice(q * DIN // 2, (q + 1) * DIN // 2)
                    eng.dma_start(ch[:, cs], x_d[m * 128:(m + 1) * 128, cs])
                x_chunks[m] = ch

            w_chunks = {}

            def load_w_chunk(n, c):
                ch = wchunk_pool.tile([128, DIN], BF16, tag="wchunk")
                nc.gpsimd.dma_start(
                    ch[:], w_d[n * NSL + c * 128:n * NSL + (c + 1) * 128, :]
                )
                w_chunks[(n, c)] = ch

            # bias first on the SWDGE queue (tiny cast-DMA), then the bulk
            # chunk loads; B/A ride the sync HW queue as f32 (the SWDGE
            # software gather takes ~45us and stalled the PE for 25us)
            a_aug = const_pool.tile([128, DOUT], BF16)
            for p0 in range(R, 128, 32):
                nc.gpsimd.memset(a_aug[p0:p0 + 32, :], 0.0)
            nc.gpsimd.dma_start(a_aug[R:R + 1, :], bias_d[None, :])
            load_x_chunk(0)
            load_x_chunk(1)
            # B/A gathers: quartered across SWDGE rings so they finish in
            # ~1/4 the time (a single gather ran ~25-45us and stalled the
            # PE before the first u batch); issued before the W chunks.
            b_all_nat = b_d.rearrange("(k p) r -> p k r", p=128)
            a_all_nat = a_d.rearrange("(o p) r -> p o r", p=128)
            b_nat32 = const_pool.tile([128, KT, R], F32)
            a_nat32 = const_pool.tile([128, DOUT // 128, R], F32)
            for q in range(4):
                ks = slice(q * KT // 4, (q + 1) * KT // 4)
                nc.gpsimd.dma_start(b_nat32[:, ks, :], b_all_nat[:, ks, :])
            for q in range(4):
                ks = slice(q * 8, (q + 1) * 8)
                nc.gpsimd.dma_start(a_nat32[:, ks, :], a_all_nat[:, ks, :])
            for c in range(CPS):
                load_w_chunk(0, c)

            make_identity(nc, ident_f32[:])
            nc.gpsimd.memset(u_aug[:], 0.0)
            nc.gpsimd.memset(u_aug[R:R + 1, :], 1.0)

            w_t = [
                wT_pool.tile([128, KTB, NSL], BF16, tag="wt", name="wt0"),
                wT_pool.tile([128, KTB, NSL], BF16, tag="wt", name="wt1"),
            ]
            w8_t = [
                wT_pool.tile([128, KT8, NSL], FP8, tag="wt8", name="w8t0"),
                wT_pool.tile([128, KT8, NSL], FP8, tag="wt8", name="w8t1"),
            ]

            def split_ranges(b0, bs):
                """Split block range [b0, b0+bs) at the KTB dtype boundary."""
                if b0 >= KTB or b0 + bs <= KTB:
                    return [(b0, b0 + bs)]
                return [(b0, KTB), (KTB, b0 + bs)]

            def w_transpose_run(n, c, k0, nk):
                """Transpose nk k-blocks (k indices k0..) of chunk c of slice
                n into w_t (bf16, k<KTB) / w8_t (fp8, k>=KTB), batching one
                full PSUM bank per copy; copies apply the x256 fp8 scale."""
                ch = w_chunks[(n, c)]
                f32 = ch.dtype == F32
                bs = 4 if f32 else 8
                idn = ident_f32 if f32 else ident
                col0 = c * 128
                for b0 in range(k0, k0 + nk, bs):
                    pt = tpsum_pool.tile(
                        [128, bs, 128], F32 if f32 else BF16, tag="t"
                    )
                    for j in range(bs):
                        nc.tensor.transpose(
                            pt[:, j, :],
                            ch[:, (b0 + j) * 128:(b0 + j + 1) * 128],
                            idn[:],
                        )
                    for r0, r1 in split_ranges(b0, bs):
                        if r0 >= KTB:
                            dst = w8_t[n % 2][:, r0 - KTB:r1 - KTB,
                                              col0:col0 + 128]
                        else:
                            dst = w_t[n % 2][:, r0:r1, col0:col0 + 128]
                        tcopy_scaled(dst, pt[:, r0 - b0:r1 - b0, :], SW)

            def x_transpose_run(m):
                """Transpose all KT k-blocks of x m-chunk: k<KTB into x_t
                (bf16), k>=KTB into x8_t (fp8, unscaled cast)."""
                chunk = x_chunks.pop(m)
                f32 = chunk.dtype == F32
                bs = 4 if f32 else 8
                idn = ident_f32 if f32 else ident
                col0 = m * 128
                for b0 in range(0, KT, bs):
                    pt = tpsum_pool.tile(
                        [128, bs, 128], F32 if f32 else BF16, tag="t"
                    )
                    for j in range(bs):
                        nc.tensor.transpose(
                            pt[:, j, :],
                            chunk[:, (b0 + j) * 128:(b0 + j + 1) * 128],
                            idn[:],
                        )
                    for r0, r1 in split_ranges(b0, bs):
                        if r0 >= KTB:
                            dst = x8_t[:, r0 - KTB:r1 - KTB, col0:col0 + 128]
                        else:
                            dst = x_t[:, r0:r1, col0:col0 + 128]
                        tcopy(dst, pt[:, r0 - b0:r1 - b0, :])

            def u_batch(mc):
                """u[:, mc*256:(mc+1)*256] = 256*(x @ B).T for 2 m-tiles."""
                cols = slice(mc * 256, (mc + 1) * 256)
                up = ypsum_pool.tile([R, 256], F32, tag="y", name="up")
                for k in range(KTB):
                    nc.tensor.matmul(
                        up[:], b_all[:, k, :], x_t[:, k, cols],
                        start=(k == 0), stop=False,
                    )
                for kk in range(KT8 // 2):
                    nc.tensor.matmul(
                        up[:],
                        b8_all[:, 2 * kk:2 * kk + 2, :],
                        x8_t[:, 2 * kk:2 * kk + 2, cols],
                        start=False, stop=(kk == KT8 // 2 - 1),
                        perf_mode=DR, skip_group_check=True,
                    )
                tcopy(u_aug[0:R, cols], up[:])

            def main_tile(n, m, before_lora=None):
                """One [128 tok, 512 out] output tile: bf16 k-loop + fp8-DR
                k-loop + augmented LoRA matmul, then scaled copy-out + DMA.
                before_lora emits extra PE work between the k-loop and the
                LoRA matmul (startup uses this to cover the A gather)."""
                cur = w_t[n % 2]
                cur8 = w8_t[n % 2]
                yp = ypsum_pool.tile([128, NSL], F32, tag="y")
                for k in range(KTB):
                    nc.tensor.matmul(
                        yp[:], x_t[:, k, m * 128:(m + 1) * 128], cur[:, k, :],
                        start=(k == 0), stop=False,
                    )
                for kk in range(KT8 // 2):
                    nc.tensor.matmul(
                        yp[:],
                        x8_t[:, 2 * kk:2 * kk + 2, m * 128:(m + 1) * 128],
                        cur8[:, 2 * kk:2 * kk + 2, :],
                        start=False, stop=False,
                        perf_mode=DR, skip_group_check=True,
                    )
                if before_lora is not None:
                    before_lora()
                nc.tensor.matmul(
                    yp[:],
                    u_aug[:, m * 128:(m + 1) * 128],
                    a_aug[:, n * NSL:(n + 1) * NSL],
                    start=False, stop=True, skip_group_check=True,
                )
                y_sb = y_pool.tile([128, NSL], F32, tag="ysb")
                nc.scalar.mul(y_sb[:], yp[:], 1.0 / SW)
                nc.sync.dma_start(
                    y_d[m * 128:(m + 1) * 128, n * NSL:(n + 1) * NSL],
                    y_sb[:],
                )

            # ---- startup: x m=0,1 + W slice 0 + B/A constants ----
            nc.vector.tensor_scalar_mul(
                a_aug[R:R + 1, :], a_aug[R:R + 1, :], SW
            )
            x_transpose_run(0)
            load_x_chunk(2)
            x_transpose_run(1)
            load_x_chunk(3)
            load_w_chunk(1, 0)
            w_transpose_run(0, 0, 0, 16)
            w_transpose_run(0, 0, 16, 16)
            w_transpose_run(0, 1, 0, 16)
            # B: x256 scale + bf16 cast, + fp8 cast for the fp8 k-range
            # (emitted mid-wT0 so a late B gather can't block the wT0
            # PSUM-copy chain on DVE/ACT)
            nc.vector.tensor_scalar_mul(b_all[:], b_nat32[:], SW)
            nc.scalar.copy(b8_all[:], b_all[:, KTB:, :])
            w_transpose_run(0, 1, 16, 16)
            w_transpose_run(0, 2, 0, 16)
            w_transpose_run(0, 2, 16, 16)
            w_transpose_run(0, 3, 0, 16)
            w_transpose_run(0, 3, 16, 16)
            load_w_chunk(1, 1)
            # W slice-1 chunk-0 transposes: PE cover for the B gather+casts
            w_transpose_run(1, 0, 0, 16)
            w_transpose_run(1, 0, 16, 16)
            u_batch(0)

            def build_a_aug():
                # A_aug rows 0:R = A.T (f32 transposes, cast on copy-out);
                # runs inside tile (0,0) so the k-loop covers the A gather
                for o in range(DOUT // 128):
                    pt = tpsum_pool.tile([R, 128], F32, tag="t")
                    nc.tensor.transpose(pt[:], a_nat32[:, o, :], ident_f32[:])
                    nc.vector.tensor_copy(
                        a_aug[0:R, o * 128:(o + 1) * 128], pt[:]
                    )

            main_tile(0, 0, before_lora=build_a_aug)

            # ---- slices 0 and 1 interleaved in half-slices so the tail x
            # chunks (m=4..7) get ~100us of DMA slack instead of ~10us ----
            # phase A: tiles (0, 1..3) + slice-1 transposes
            for m in range(1, 4):
                if m == 1:
                    load_w_chunk(1, 2)
                    load_w_chunk(1, 3)
                main_tile(0, m)
                w_transpose_run(1, m, 0, 16)
                w_transpose_run(1, m, 16, 16)
                if m == 1:
                    x_transpose_run(2)
                    load_x_chunk(4)
                    x_transpose_run(3)
                    load_x_chunk(5)
                    u_batch(1)
            # phase B: tiles (1, 0..3) + slice-2 chunk loads
            for m in range(4):
                load_w_chunk(2, m)
                main_tile(1, m)
                if m == 0:
                    x_transpose_run(4)
                    load_x_chunk(6)
                if m == 1:
                    x_transpose_run(5)
                    load_x_chunk(7)
                    u_batch(2)
            # phase C: tiles (0, 4..7)
            for m in range(4, MT):
                main_tile(0, m)
                if m == 4:
                    x_transpose_run(6)
                if m == 5:
                    x_transpose_run(7)
                    u_batch(3)
            # phase D: tiles (1, 4..7) + slice-2 transposes
            for m in range(4, MT):
                main_tile(1, m)
                w_transpose_run(2, m - 4, 0, 16)
                w_transpose_run(2, m - 4, 16, 16)

            # ---- main loop over remaining output-feature slices ----
            for n in range(2, NT):
                for m in range(MT):
                    if n + 1 < NT:
                        # chunk c of slice n+1 is consumed at m=2c and 2c+1;
                        # load it one m-iteration ahead (c=0 at m=0).
                        if m == 0:
                            load_w_chunk(n + 1, 0)
                        if m % 2 == 1 and (m + 1) // 2 < CPS:
                            load_w_chunk(n + 1, (m + 1) // 2)
                    main_tile(n, m)
                    if n + 1 < NT:
                        # 16 transposes of slice n+1 after each m's matmuls
                        w_transpose_run(n + 1, m // 2, (m % 2) * 16, 16)

    nc.compile()
    return nc


def _get_nc():
    global _cached
    if _cached is None:
        _cached = _build()
    return _cached


def kernel(x, weight, bias, A, B, _trace=False):
    x = np.ascontiguousarray(np.asarray(x, dtype=np.float32)).reshape(-1, DIN)
    weight = np.ascontiguousarray(np.asarray(weight, dtype=np.float32))
    bias = np.ascontiguousarray(np.asarray(bias, dtype=np.float32))
    A = np.ascontiguousarray(np.asarray(A, dtype=np.float32))
    B = np.ascontiguousarray(np.asarray(B, dtype=np.float32))

    nc = _get_nc()
    in_maps = [
        {
            "x": np.ascontiguousarray(x[c * TOK:(c + 1) * TOK]),
            "weight": weight,
            "bias": bias,
            "A": A,
            "B": B,
        }
        for c in range(N_CORES)
    ]
    # The axon trn2 runtime very occasionally hard-faults a core
    # (NRT_EXEC_UNIT_UNRECOVERABLE) or silently corrupts a DMA on a run.
    # The kernel is bitwise-deterministic when healthy, so run until two
    # executions agree exactly (normally exactly 2 runs), retrying past
    # crashes and one-off corruptions.
    import time as _time

    outs = []
    last_exc = None
    for attempt in range(5):
        try:
            res = run_bass_kernel_spmd(
                nc, in_maps, core_ids=list(range(N_CORES)), trace=_trace
            )
            kernel.last_result = res
            y = np.concatenate(
                [np.asarray(res.results[c]["out"]) for c in range(N_CORES)],
                axis=0,
            ).reshape(4, 2048, DOUT)
        except Exception as exc:  # noqa: BLE001
            last_exc = exc
            _time.sleep(10)
            continue
        for prev in outs:
            if np.array_equal(prev, y):
                return y
        outs.append(y)
    if outs:
        return outs[-1]
    raise last_exc


kernel.last_result = None
